# revision 18
# baseline (speedup 1.0000x reference)
"""Trainium2 Bass kernel for nn_BaseGraphEncoder (gnn_message_passing).

Computation (per batch b):
    h0 = max_k x[:, idx[b,n,k]]            (gather + K-max, "local maxpool")
    h1 = h0 @ W1 + b1
    r  = relu(Wc1 @ h1 + bc1)              (conv1d k=1)
    h2 = max_k r[:, idx[b,n,k]]            (second local maxpool, same idx)
    h3 = h2 @ W2 + b2
    out = Wc2 @ h3 + bc2                   (conv1d k=1, 1024 out channels)

Sharding: data-parallel over batch B=16 across 8 NeuronCores (2 batches/core).
The kNN gather is batch-local so no cross-core communication is needed.

Device strategy per core (2 batches):
  - The gather is gpsimd.dma_gather (SWDGE indexed DMA) from point-major
    (N, 64) f32 = 256B rows in DRAM. Pool-engine descriptor generation
    (~7.9 ns/idx, ~32.3 us per 4096-idx instruction) is the hard bottleneck
    (~1.035 ms/core total); everything else hides under it:
      * 4096-idx chunks keep each instruction's descriptor push (257/DMA)
        within the SWDGE ring so consecutive gathers run back-to-back
        (8192-idx chunks stall ~10 us each waiting for ring drain).
      * (multi-queue SWDGE rotation was tried and crashes this runtime's
        ucode -- NRT assigns the Pool engine a single queue; all gathers
        stay on queue 0.)
      * the gather list is ordered in n-major blocks of 256 points
        (k-major within a block), so each chunk fully reduces one block
        via a log2 tree of DVE maxes, letting the per-block transposes,
        matmuls, conv2 and output DMA pipeline inside the gather shadow.
      * the two batches are emitted phase-interleaved (L1 b0, b1 -> L2 b0,
        b1) so the Pool engine never idles at batch/layer boundaries.
  - All matmuls run in fp32r (PE 1 cycle/row at >=256 cols vs 4 for fp32).
  - Bias (+relu) is applied by the ACT engine during PSUM->SBUF eviction,
    including the conv2 output (no bias-preload matmuls).

Block/index layout. Chunk c of a batch covers points [256c, 256c+256);
within the chunk, list position i = j*256 + dn reads neighbor j of point
dn (dn in [0,256)). The gather writes list position i to partition i%128,
free slot i//128, so point dn lands on partition dn%128 at free slot
a = 2j + dn//128. Pairing slots (a, a+16) repeatedly max-reduces over j
(a tree of [128,16,64] -> [128,2,64] DVE tensor_tensor ops) and the final
[128, 2, 64] drops directly into the point-major acc[:, 2c:2c+2, :]
(acc[p, q, :] = features of point q*128 + p).
"""

import sys

if "/opt/trn_rl_repo" not in sys.path:
    sys.path.insert(0, "/opt/trn_rl_repo")

import numpy as np

import concourse.bacc as bacc
import concourse.bass as bass
import concourse.mybir as mybir
import concourse.tile as tile
from concourse._compat import get_trn_type
from concourse.bass_utils import run_bass_kernel_spmd

B, C, N, K = 16, 64, 2048, 16
NCORES = 8
BPC = B // NCORES  # batches per core
D1, D2, DOUT = 64, 128, 1024  # hidden dims
NIDX = N * K  # 32768 gather indices per batch per layer
NCHUNK = 8
CH_IDX = NIDX // NCHUNK  # 4096 indices per chunk = 256 points x 16 neighbors
NBLK = N // 128  # 16 point-groups of 128 in acc layout
F32 = mybir.dt.float32
F32R = mybir.dt.float32r
I16 = mybir.dt.int16

_compiled = None


def _build_nc(reps=1, gbufs=6):
    nc = bacc.Bacc(
        get_trn_type() or "TRN2",
        target_bir_lowering=False,
        debug=False,
    )

    xt_d = nc.dram_tensor("xt", [BPC, N, C], F32, kind="ExternalInput")
    idx_d = nc.dram_tensor("idx16", [BPC, 128, NIDX // 16], I16, kind="ExternalInput")
    w1_d = nc.dram_tensor("W1", [C, D1], F32R, kind="ExternalInput")
    wc1t_d = nc.dram_tensor("Wc1T", [D1, D1], F32R, kind="ExternalInput")
    w2_d = nc.dram_tensor("W2", [D1, D2], F32R, kind="ExternalInput")
    wc2t_d = nc.dram_tensor("Wc2T", [D2, DOUT], F32R, kind="ExternalInput")
    b1_d = nc.dram_tensor("b1", [D1, 1], F32, kind="ExternalInput")
    bc1_d = nc.dram_tensor("bc1", [D1, 1], F32, kind="ExternalInput")
    b2_d = nc.dram_tensor("b2", [D2, 1], F32, kind="ExternalInput")
    bc2_d = nc.dram_tensor("bc2", [128, DOUT // 128], F32, kind="ExternalInput")
    id_d = nc.dram_tensor("ident", [128, 128], F32, kind="ExternalInput")
    out_d = nc.dram_tensor("out", [BPC, DOUT, N], F32, kind="ExternalOutput")

    with tile.TileContext(nc) as tc:
        with (
            tc.tile_pool(name="consts", bufs=1) as consts,
            tc.tile_pool(name="gpool", bufs=gbufs) as gpool,
            tc.tile_pool(name="mpool", bufs=2) as mpool,
            tc.tile_pool(name="accpool", bufs=2) as accpool,
            tc.tile_pool(name="idxpool", bufs=1) as idxpool,
            tc.tile_pool(name="featpool", bufs=2) as featpool,
            tc.tile_pool(name="h3pool", bufs=2) as h3pool,
            tc.tile_pool(name="rpmpool", bufs=2) as rpmpool,
            tc.tile_pool(name="outpool", bufs=4) as outpool,
            tc.tile_pool(name="pst", bufs=2, space="PSUM") as pst,
            tc.tile_pool(name="psm", bufs=2, space="PSUM") as psm,
            tc.tile_pool(name="pso", bufs=4, space="PSUM") as pso,
            tc.tile_pool(name="drampool", bufs=2, space="DRAM") as drampool,
        ):
            # ---- idx first (first gather waits on it), then weights ----
            idx_sbs = {}
            IQ = NIDX // 16 // 4
            for b in range(BPC):
                idx_sb = idxpool.tile([128, NIDX // 16], I16, tag=f"idx{b}")
                for s in range(4):
                    nc.sync.dma_start(
                        idx_sb[:, s * IQ : (s + 1) * IQ],
                        idx_d[b, :, s * IQ : (s + 1) * IQ],
                    )
                idx_sbs[b] = idx_sb
            w1_sb = consts.tile([C, D1], F32R, tag="w1")
            wc1t_sb = consts.tile([D1, D1], F32R, tag="wc1t")
            w2_sb = consts.tile([D1, D2], F32R, tag="w2")
            wc2t_sb = consts.tile([D2, DOUT], F32R, tag="wc2t")
            b1_sb = consts.tile([D1, 1], F32, tag="b1")
            bc1_sb = consts.tile([D1, 1], F32, tag="bc1")
            b2_sb = consts.tile([D2, 1], F32, tag="b2")
            bc2_sb = consts.tile([128, DOUT // 128], F32, tag="bc2")
            id_sb = consts.tile([128, 128], F32, tag="ident")
            nc.sync.dma_start(w1_sb, w1_d[:])
            nc.sync.dma_start(wc1t_sb, wc1t_d[:])
            nc.sync.dma_start(w2_sb, w2_d[:])
            nc.sync.dma_start(wc2t_sb, wc2t_d[:])
            nc.sync.dma_start(b1_sb, b1_d[:])
            nc.sync.dma_start(bc1_sb, bc1_d[:])
            nc.sync.dma_start(b2_sb, b2_d[:])
            nc.sync.dma_start(bc2_sb, bc2_d[:])
            nc.sync.dma_start(id_sb, id_d[:])

            def emit_gather_chunk(src_ap, idx_sb, acc3, ch):
                """Gather chunk ch (points [256ch, 256ch+256) x 16 nbrs) and
                tree-max it into acc3[:, 2ch:2ch+2, :]."""
                g = gpool.tile([128, CH_IDX // 128, C], F32, tag="g")
                nc.gpsimd.dma_gather(
                    g,
                    src_ap,
                    idx_sb[:, ch * (CH_IDX // 16) : (ch + 1) * (CH_IDX // 16)],
                    CH_IDX,
                    CH_IDX,
                    C,
                    single_packet=False,
                )
                m1 = mpool.tile([128, 16, C], F32, tag="m1")
                nc.vector.tensor_tensor(
                    m1, g[:, 0:16, :], g[:, 16:32, :], mybir.AluOpType.max
                )
                m2 = mpool.tile([128, 8, C], F32, tag="m2")
                nc.vector.tensor_tensor(
                    m2, m1[:, 0:8, :], m1[:, 8:16, :], mybir.AluOpType.max
                )
                m3 = mpool.tile([128, 4, C], F32, tag="m3")
                nc.vector.tensor_tensor(
                    m3, m2[:, 0:4, :], m2[:, 4:8, :], mybir.AluOpType.max
                )
                nc.vector.tensor_tensor(
                    acc3[:, 2 * ch : 2 * ch + 2, :],
                    m3[:, 0:2, :],
                    m3[:, 2:4, :],
                    mybir.AluOpType.max,
                )

            def emit_pm_to_fm_block(acc, fm, m):
                """Transpose point-groups 4m..4m+3 of point-major acc into
                feature-major fm[:, m*512:(m+1)*512]."""
                pt = pst.tile([128, 512], F32, tag="pt")
                for qq in range(4):
                    q = 4 * m + qq
                    nc.tensor.transpose(
                        pt[:C, qq * 128 : (qq + 1) * 128],
                        acc[:, q * C : (q + 1) * C],
                        id_sb,
                    )
                nc.vector.tensor_copy(fm[:, m * 512 : (m + 1) * 512], pt[:C, :])

            def emit_mid_block(b, m, acc, fm, h1, r, r_pm, rt):
                """After chunks 2m, 2m+1: h0T block -> h1 -> r -> r_pm -> rt."""
                emit_pm_to_fm_block(acc, fm, m)
                sl = slice(m * 512, (m + 1) * 512)
                pm = psm.tile([128, 512], F32, tag="pm")
                nc.tensor.matmul(pm[:D1, :], w1_sb, fm[:, sl])
                nc.scalar.activation(
                    h1[:, sl],
                    pm[:D1, :],
                    mybir.ActivationFunctionType.Identity,
                    bias=b1_sb,
                )
                pm2 = psm.tile([128, 512], F32, tag="pm")
                nc.tensor.matmul(pm2[:D1, :], wc1t_sb, h1[:, sl])
                nc.scalar.activation(
                    r[:, sl],
                    pm2[:D1, :],
                    mybir.ActivationFunctionType.Relu,
                    bias=bc1_sb,
                )
                # r block back to point-major rows and out to the DRAM scratch
                pt2 = pst.tile([128, 512], F32, tag="pt")
                for qq in range(4):
                    q = 4 * m + qq
                    nc.tensor.transpose(
                        pt2[:, qq * C : (qq + 1) * C],
                        r[:, q * 128 : (q + 1) * 128].bitcast(F32),
                        id_sb[:C, :C],
                    )
                nc.vector.tensor_copy(r_pm[:, m * 256 : (m + 1) * 256], pt2[:, :256])
                nc.sync.dma_start(
                    rt[4 * m : 4 * m + 4].rearrange("q p c -> p q c"),
                    r_pm[:, m * 256 : (m + 1) * 256].rearrange(
                        "p (q c) -> p q c", c=C
                    ),
                )

            def emit_tail_block(b, acc, fm2, h3, g0, ng):
                """After L2 chunks covering groups g0..g0+ng: h2T cols ->
                h3 -> conv2 -> out (wd = ng*128 columns)."""
                wd = ng * 128
                sl = slice(g0 * 128, g0 * 128 + wd)
                pt = pst.tile([128, 512], F32, tag="pt")
                for qq in range(ng):
                    q = g0 + qq
                    nc.tensor.transpose(
                        pt[:C, qq * 128 : (qq + 1) * 128],
                        acc[:, q * C : (q + 1) * C],
                        id_sb,
                    )
                nc.vector.tensor_copy(fm2[:, sl], pt[:C, :wd])
                pm = psm.tile([128, 512], F32, tag="pm")
                nc.tensor.matmul(pm[:, :wd], w2_sb, fm2[:, sl])
                nc.scalar.activation(
                    h3[:, sl],
                    pm[:, :wd],
                    mybir.ActivationFunctionType.Identity,
                    bias=b2_sb,
                )
                for dc in range(8):
                    po = pso.tile([128, 512], F32, tag="po")
                    nc.tensor.matmul(
                        po[:, :wd], wc2t_sb[:, dc * 128 : (dc + 1) * 128], h3[:, sl]
                    )
                    osb = outpool.tile([128, 512], F32, tag="osb")
                    if dc % 2 == 0:
                        nc.scalar.activation(
                            osb[:, :wd],
                            po[:, :wd],
                            mybir.ActivationFunctionType.Identity,
                            bias=bc2_sb[:, dc : dc + 1],
                        )
                    else:
                        nc.vector.tensor_scalar_add(
                            osb[:, :wd], po[:, :wd], bc2_sb[:, dc : dc + 1]
                        )
                    nc.sync.dma_start(
                        out_d[b, dc * 128 : (dc + 1) * 128, sl], osb[:, :wd]
                    )

            def emit_all():
                rts = {}
                # ---------- layer 1 (gathers on Pool; block compute beneath) --
                for b in range(BPC):
                    acc1 = accpool.tile([128, NBLK, C], F32, tag="acc")
                    fm = featpool.tile([C, N], F32R, tag="fm")
                    h1 = featpool.tile([D1, N], F32R, tag="h1")
                    r = featpool.tile([D1, N], F32R, tag="r")
                    r_pm = rpmpool.tile([128, NBLK * C], F32, tag="rpm")
                    rt = drampool.tile([N // 128, 128, C], F32, tag="rt")
                    acc1f = acc1.rearrange("p q c -> p (q c)")
                    for m in range(4):
                        for cc in (2 * m, 2 * m + 1):
                            emit_gather_chunk(xt_d[b], idx_sbs[b], acc1, cc)
                        emit_mid_block(b, m, acc1f, fm, h1, r, r_pm, rt)
                    rts[b] = rt

                # ---------- layer 2 ----------
                for b in range(BPC):
                    acc2 = accpool.tile([128, NBLK, C], F32, tag="acc")
                    fm2 = featpool.tile([C, N], F32R, tag="fm")
                    h3 = h3pool.tile([D2, N], F32R, tag="h3")
                    acc2f = acc2.rearrange("p q c -> p (q c)")
                    src = rts[b].rearrange("q p c -> (q p) c")
                    for m in range(4):
                        split = b == BPC - 1 and m == 3
                        for cc in (2 * m, 2 * m + 1):
                            emit_gather_chunk(src, idx_sbs[b], acc2, cc)
                            if split:
                                emit_tail_block(b, acc2f, fm2, h3, 2 * cc, 2)
                        if not split:
                            emit_tail_block(b, acc2f, fm2, h3, 4 * m, 4)

            if reps == 1:
                emit_all()
            else:
                with tc.For_i(0, reps, 1):
                    emit_all()

    nc.compile()
    return nc


def _get_nc():
    global _compiled
    if _compiled is None:
        _compiled = _build_nc()
    return _compiled


def _prep_inputs(x, idx, W1, b1, Wc1, bc1, W2, b2, Wc2, bc2):
    """Host-side sharding + layout marshalling -> per-core in_maps."""
    x = np.asarray(x, np.float32)
    idx = np.asarray(idx)
    xt = np.ascontiguousarray(x.transpose(0, 2, 1))  # (B, N, C) point-major rows

    # batch-local indices (reference guarantees idx[b] in [b*N, (b+1)*N))
    local = idx.astype(np.int64) - (np.arange(B, dtype=np.int64) * N)[:, None, None]
    assert local.min() >= 0 and local.max() < N, "idx not batch-local"
    local = local.astype(np.int16)  # (B, N, K)

    # gather list: n-major blocks of 256 points, k-major within the block:
    # chunk c, position i = j*256 + dn  ->  local[b, 256c + dn, j]
    PB = CH_IDX // K  # 256 points per block
    blk = local.reshape(B, NCHUNK, PB, K)  # (B, c, dn, j)
    lst = np.ascontiguousarray(blk.transpose(0, 1, 3, 2)).reshape(B, NIDX)
    # dma_gather wrap: W[p, s] = lst[s*16 + p]
    wrapped = lst.reshape(B, NIDX // 16, 16).transpose(0, 2, 1)  # (B, 16, NIDX/16)
    wrapped = np.ascontiguousarray(
        np.tile(wrapped, (1, 8, 1))
    )  # replicate to 128 partitions

    common = {
        "W1": np.ascontiguousarray(np.asarray(W1, np.float32)),
        "Wc1T": np.ascontiguousarray(np.asarray(Wc1, np.float32).T),
        "W2": np.ascontiguousarray(np.asarray(W2, np.float32)),
        "Wc2T": np.ascontiguousarray(np.asarray(Wc2, np.float32).T),
        "b1": np.asarray(b1, np.float32).reshape(D1, 1),
        "bc1": np.asarray(bc1, np.float32).reshape(D1, 1),
        "b2": np.asarray(b2, np.float32).reshape(D2, 1),
        "bc2": np.ascontiguousarray(
            np.asarray(bc2, np.float32).reshape(DOUT // 128, 128).T
        ),
        "ident": np.eye(128, dtype=np.float32),
    }
    in_maps = []
    for c in range(NCORES):
        bs = [BPC * c + j for j in range(BPC)]
        m = dict(common)
        m["xt"] = np.ascontiguousarray(xt[bs])
        m["idx16"] = np.ascontiguousarray(wrapped[bs])
        in_maps.append(m)
    return in_maps


def kernel(_trace=False, _trace_kwargs=None, **inputs):
    nc = _get_nc()
    in_maps = _prep_inputs(**inputs)
    res = run_bass_kernel_spmd(
        nc,
        in_maps,
        list(range(NCORES)),
        trace=_trace,
        **(_trace_kwargs or {}),
    )
    out = np.empty((B, DOUT, N), np.float32)
    for c in range(NCORES):
        for j in range(BPC):
            out[BPC * c + j] = res.results[c]["out"][j]
    if _trace:
        return out, res
    return out


# revision 19
# speedup vs baseline: 1.0002x; 1.0002x over previous
"""Trainium2 Bass kernel for nn_BaseGraphEncoder (gnn_message_passing).

Computation (per batch b):
    h0 = max_k x[:, idx[b,n,k]]            (gather + K-max, "local maxpool")
    h1 = h0 @ W1 + b1
    r  = relu(Wc1 @ h1 + bc1)              (conv1d k=1)
    h2 = max_k r[:, idx[b,n,k]]            (second local maxpool, same idx)
    h3 = h2 @ W2 + b2
    out = Wc2 @ h3 + bc2                   (conv1d k=1, 1024 out channels)

Sharding: data-parallel over batch B=16 across 8 NeuronCores (2 batches/core).
The kNN gather is batch-local so no cross-core communication is needed.

Device strategy per core (2 batches):
  - The gather is gpsimd.dma_gather (SWDGE indexed DMA) from point-major
    (N, 64) f32 = 256B rows in DRAM. Pool-engine descriptor generation
    (~7.9 ns/idx, ~32.3 us per 4096-idx instruction) is the hard bottleneck
    (~1.035 ms/core total); everything else hides under it:
      * 4096-idx chunks keep each instruction's descriptor push (257/DMA)
        within the SWDGE ring so consecutive gathers run back-to-back
        (8192-idx chunks stall ~10 us each waiting for ring drain).
      * (multi-queue SWDGE rotation was tried and crashes this runtime's
        ucode -- NRT assigns the Pool engine a single queue; all gathers
        stay on queue 0.)
      * the gather list is ordered in n-major blocks of 256 points
        (k-major within a block), so each chunk fully reduces one block
        via a log2 tree of DVE maxes, letting the per-block transposes,
        matmuls, conv2 and output DMA pipeline inside the gather shadow.
      * the two batches are emitted phase-interleaved (L1 b0, b1 -> L2 b0,
        b1) so the Pool engine never idles at batch/layer boundaries.
  - All matmuls run in fp32r (PE 1 cycle/row at >=256 cols vs 4 for fp32).
  - Bias (+relu) is applied by the ACT engine during PSUM->SBUF eviction,
    including the conv2 output (no bias-preload matmuls).

Block/index layout. Chunk c of a batch covers points [256c, 256c+256);
within the chunk, list position i = j*256 + dn reads neighbor j of point
dn (dn in [0,256)). The gather writes list position i to partition i%128,
free slot i//128, so point dn lands on partition dn%128 at free slot
a = 2j + dn//128. Pairing slots (a, a+16) repeatedly max-reduces over j
(a tree of [128,16,64] -> [128,2,64] DVE tensor_tensor ops) and the final
[128, 2, 64] drops directly into the point-major acc[:, 2c:2c+2, :]
(acc[p, q, :] = features of point q*128 + p).
"""

import sys

if "/opt/trn_rl_repo" not in sys.path:
    sys.path.insert(0, "/opt/trn_rl_repo")

import numpy as np

import concourse.bacc as bacc
import concourse.bass as bass
import concourse.mybir as mybir
import concourse.tile as tile
from concourse._compat import get_trn_type
from concourse.bass_utils import run_bass_kernel_spmd

B, C, N, K = 16, 64, 2048, 16
NCORES = 8
BPC = B // NCORES  # batches per core
D1, D2, DOUT = 64, 128, 1024  # hidden dims
NIDX = N * K  # 32768 gather indices per batch per layer
NCHUNK = 8
CH_IDX = NIDX // NCHUNK  # 4096 indices per chunk = 256 points x 16 neighbors
NBLK = N // 128  # 16 point-groups of 128 in acc layout
F32 = mybir.dt.float32
F32R = mybir.dt.float32r
I16 = mybir.dt.int16

_compiled = None


def _build_nc(reps=1, gbufs=6):
    nc = bacc.Bacc(
        get_trn_type() or "TRN2",
        target_bir_lowering=False,
        debug=False,
    )

    xt_d = nc.dram_tensor("xt", [BPC, N, C], F32, kind="ExternalInput")
    idx_d = nc.dram_tensor("idx16", [BPC, 128, NIDX // 16], I16, kind="ExternalInput")
    w1_d = nc.dram_tensor("W1", [C, D1], F32R, kind="ExternalInput")
    wc1t_d = nc.dram_tensor("Wc1T", [D1, D1], F32R, kind="ExternalInput")
    w2_d = nc.dram_tensor("W2", [D1, D2], F32R, kind="ExternalInput")
    wc2t_d = nc.dram_tensor("Wc2T", [D2, DOUT], F32R, kind="ExternalInput")
    b1_d = nc.dram_tensor("b1", [D1, 1], F32, kind="ExternalInput")
    bc1_d = nc.dram_tensor("bc1", [D1, 1], F32, kind="ExternalInput")
    b2_d = nc.dram_tensor("b2", [D2, 1], F32, kind="ExternalInput")
    bc2_d = nc.dram_tensor("bc2", [128, DOUT // 128], F32, kind="ExternalInput")
    id_d = nc.dram_tensor("ident", [128, 128], F32, kind="ExternalInput")
    out_d = nc.dram_tensor("out", [BPC, DOUT, N], F32, kind="ExternalOutput")

    with tile.TileContext(nc) as tc:
        with (
            tc.tile_pool(name="consts", bufs=1) as consts,
            tc.tile_pool(name="gpool", bufs=gbufs) as gpool,
            tc.tile_pool(name="mpool", bufs=2) as mpool,
            tc.tile_pool(name="accpool", bufs=2) as accpool,
            tc.tile_pool(name="idxpool", bufs=1) as idxpool,
            tc.tile_pool(name="featpool", bufs=2) as featpool,
            tc.tile_pool(name="h3pool", bufs=2) as h3pool,
            tc.tile_pool(name="rpmpool", bufs=2) as rpmpool,
            tc.tile_pool(name="outpool", bufs=4) as outpool,
            tc.tile_pool(name="pst", bufs=2, space="PSUM") as pst,
            tc.tile_pool(name="psm", bufs=2, space="PSUM") as psm,
            tc.tile_pool(name="pso", bufs=4, space="PSUM") as pso,
            tc.tile_pool(name="drampool", bufs=2, space="DRAM") as drampool,
        ):
            # ---- idx first (first gather waits on it), then weights ----
            idx_sbs = {}
            IQ = NIDX // 16 // 4
            for b in range(BPC):
                idx_sb = idxpool.tile([128, NIDX // 16], I16, tag=f"idx{b}")
                for s in range(4):
                    nc.sync.dma_start(
                        idx_sb[:, s * IQ : (s + 1) * IQ],
                        idx_d[b, :, s * IQ : (s + 1) * IQ],
                    )
                idx_sbs[b] = idx_sb
            w1_sb = consts.tile([C, D1], F32R, tag="w1")
            wc1t_sb = consts.tile([D1, D1], F32R, tag="wc1t")
            w2_sb = consts.tile([D1, D2], F32R, tag="w2")
            wc2t_sb = consts.tile([D2, DOUT], F32R, tag="wc2t")
            b1_sb = consts.tile([D1, 1], F32, tag="b1")
            bc1_sb = consts.tile([D1, 1], F32, tag="bc1")
            b2_sb = consts.tile([D2, 1], F32, tag="b2")
            bc2_sb = consts.tile([128, DOUT // 128], F32, tag="bc2")
            id_sb = consts.tile([128, 128], F32, tag="ident")
            nc.sync.dma_start(w1_sb, w1_d[:])
            nc.sync.dma_start(wc1t_sb, wc1t_d[:])
            nc.sync.dma_start(w2_sb, w2_d[:])
            nc.sync.dma_start(wc2t_sb, wc2t_d[:])
            nc.sync.dma_start(b1_sb, b1_d[:])
            nc.sync.dma_start(bc1_sb, bc1_d[:])
            nc.sync.dma_start(b2_sb, b2_d[:])
            nc.sync.dma_start(bc2_sb, bc2_d[:])
            nc.sync.dma_start(id_sb, id_d[:])

            def emit_gather_chunk(src_ap, idx_sb, acc3, ch):
                """Gather chunk ch (points [256ch, 256ch+256) x 16 nbrs) and
                tree-max it into acc3[:, 2ch:2ch+2, :]."""
                g = gpool.tile([128, CH_IDX // 128, C], F32, tag="g")
                nc.gpsimd.dma_gather(
                    g,
                    src_ap,
                    idx_sb[:, ch * (CH_IDX // 16) : (ch + 1) * (CH_IDX // 16)],
                    CH_IDX,
                    CH_IDX,
                    C,
                    single_packet=False,
                )
                m1 = mpool.tile([128, 16, C], F32, tag="m1")
                nc.vector.tensor_tensor(
                    m1, g[:, 0:16, :], g[:, 16:32, :], mybir.AluOpType.max
                )
                m2 = mpool.tile([128, 8, C], F32, tag="m2")
                nc.vector.tensor_tensor(
                    m2, m1[:, 0:8, :], m1[:, 8:16, :], mybir.AluOpType.max
                )
                m3 = mpool.tile([128, 4, C], F32, tag="m3")
                nc.vector.tensor_tensor(
                    m3, m2[:, 0:4, :], m2[:, 4:8, :], mybir.AluOpType.max
                )
                nc.vector.tensor_tensor(
                    acc3[:, 2 * ch : 2 * ch + 2, :],
                    m3[:, 0:2, :],
                    m3[:, 2:4, :],
                    mybir.AluOpType.max,
                )

            def emit_pm_to_fm_block(acc, fm, m):
                """Transpose point-groups 4m..4m+3 of point-major acc into
                feature-major fm[:, m*512:(m+1)*512]."""
                pt = pst.tile([128, 512], F32, tag="pt")
                for qq in range(4):
                    q = 4 * m + qq
                    nc.tensor.transpose(
                        pt[:C, qq * 128 : (qq + 1) * 128],
                        acc[:, q * C : (q + 1) * C],
                        id_sb,
                    )
                nc.vector.tensor_copy(fm[:, m * 512 : (m + 1) * 512], pt[:C, :])

            def emit_mid_block(b, m, acc, fm, h1, r, r_pm, rt):
                """After chunks 2m, 2m+1: h0T block -> h1 -> r -> r_pm -> rt."""
                emit_pm_to_fm_block(acc, fm, m)
                sl = slice(m * 512, (m + 1) * 512)
                pm = psm.tile([128, 512], F32, tag="pm")
                nc.tensor.matmul(pm[:D1, :], w1_sb, fm[:, sl])
                nc.scalar.activation(
                    h1[:, sl],
                    pm[:D1, :],
                    mybir.ActivationFunctionType.Identity,
                    bias=b1_sb,
                )
                pm2 = psm.tile([128, 512], F32, tag="pm")
                nc.tensor.matmul(pm2[:D1, :], wc1t_sb, h1[:, sl])
                nc.scalar.activation(
                    r[:, sl],
                    pm2[:D1, :],
                    mybir.ActivationFunctionType.Relu,
                    bias=bc1_sb,
                )
                # r block back to point-major rows and out to the DRAM scratch
                pt2 = pst.tile([128, 512], F32, tag="pt")
                for qq in range(4):
                    q = 4 * m + qq
                    nc.tensor.transpose(
                        pt2[:, qq * C : (qq + 1) * C],
                        r[:, q * 128 : (q + 1) * 128].bitcast(F32),
                        id_sb[:C, :C],
                    )
                nc.vector.tensor_copy(r_pm[:, m * 256 : (m + 1) * 256], pt2[:, :256])
                nc.sync.dma_start(
                    rt[4 * m : 4 * m + 4].rearrange("q p c -> p q c"),
                    r_pm[:, m * 256 : (m + 1) * 256].rearrange(
                        "p (q c) -> p q c", c=C
                    ),
                )

            def emit_tail_block(b, m, acc, fm2, h3):
                """After L2 chunks 2m, 2m+1: h2T block -> h3 -> conv2 -> out."""
                emit_pm_to_fm_block(acc, fm2, m)
                sl = slice(m * 512, (m + 1) * 512)
                pm = psm.tile([128, 512], F32, tag="pm")
                nc.tensor.matmul(pm, w2_sb, fm2[:, sl])
                nc.scalar.activation(
                    h3[:, sl],
                    pm,
                    mybir.ActivationFunctionType.Identity,
                    bias=b2_sb,
                )
                for dc in range(8):
                    po = pso.tile([128, 512], F32, tag="po")
                    nc.tensor.matmul(
                        po, wc2t_sb[:, dc * 128 : (dc + 1) * 128], h3[:, sl]
                    )
                    osb = outpool.tile([128, 512], F32, tag="osb")
                    if dc % 2 == 0:
                        nc.scalar.activation(
                            osb,
                            po,
                            mybir.ActivationFunctionType.Identity,
                            bias=bc2_sb[:, dc : dc + 1],
                        )
                    else:
                        nc.vector.tensor_scalar_add(
                            osb, po, bc2_sb[:, dc : dc + 1]
                        )
                    nc.sync.dma_start(out_d[b, dc * 128 : (dc + 1) * 128, sl], osb)

            def emit_all():
                rts = {}
                # ---------- layer 1 (gathers on Pool; block compute beneath) --
                for b in range(BPC):
                    acc1 = accpool.tile([128, NBLK, C], F32, tag="acc")
                    fm = featpool.tile([C, N], F32R, tag="fm")
                    h1 = featpool.tile([D1, N], F32R, tag="h1")
                    r = featpool.tile([D1, N], F32R, tag="r")
                    r_pm = rpmpool.tile([128, NBLK * C], F32, tag="rpm")
                    rt = drampool.tile([N // 128, 128, C], F32, tag="rt")
                    acc1f = acc1.rearrange("p q c -> p (q c)")
                    for m in range(4):
                        for cc in (2 * m, 2 * m + 1):
                            emit_gather_chunk(xt_d[b], idx_sbs[b], acc1, cc)
                        emit_mid_block(b, m, acc1f, fm, h1, r, r_pm, rt)
                    rts[b] = rt

                # ---------- layer 2 ----------
                for b in range(BPC):
                    acc2 = accpool.tile([128, NBLK, C], F32, tag="acc")
                    fm2 = featpool.tile([C, N], F32R, tag="fm")
                    h3 = h3pool.tile([D2, N], F32R, tag="h3")
                    acc2f = acc2.rearrange("p q c -> p (q c)")
                    src = rts[b].rearrange("q p c -> (q p) c")
                    for m in range(4):
                        for cc in (2 * m, 2 * m + 1):
                            emit_gather_chunk(src, idx_sbs[b], acc2, cc)
                        emit_tail_block(b, m, acc2f, fm2, h3)

            if reps == 1:
                emit_all()
            else:
                with tc.For_i(0, reps, 1):
                    emit_all()

    nc.compile()
    return nc


def _get_nc():
    global _compiled
    if _compiled is None:
        _compiled = _build_nc()
    return _compiled


def _prep_inputs(x, idx, W1, b1, Wc1, bc1, W2, b2, Wc2, bc2):
    """Host-side sharding + layout marshalling -> per-core in_maps."""
    x = np.asarray(x, np.float32)
    idx = np.asarray(idx)
    xt = np.ascontiguousarray(x.transpose(0, 2, 1))  # (B, N, C) point-major rows

    # batch-local indices (reference guarantees idx[b] in [b*N, (b+1)*N))
    local = idx.astype(np.int64) - (np.arange(B, dtype=np.int64) * N)[:, None, None]
    assert local.min() >= 0 and local.max() < N, "idx not batch-local"
    local = local.astype(np.int16)  # (B, N, K)

    # gather list: n-major blocks of 256 points, k-major within the block:
    # chunk c, position i = j*256 + dn  ->  local[b, 256c + dn, j]
    PB = CH_IDX // K  # 256 points per block
    blk = local.reshape(B, NCHUNK, PB, K)  # (B, c, dn, j)
    lst = np.ascontiguousarray(blk.transpose(0, 1, 3, 2)).reshape(B, NIDX)
    # dma_gather wrap: W[p, s] = lst[s*16 + p]
    wrapped = lst.reshape(B, NIDX // 16, 16).transpose(0, 2, 1)  # (B, 16, NIDX/16)
    wrapped = np.ascontiguousarray(
        np.tile(wrapped, (1, 8, 1))
    )  # replicate to 128 partitions

    common = {
        "W1": np.ascontiguousarray(np.asarray(W1, np.float32)),
        "Wc1T": np.ascontiguousarray(np.asarray(Wc1, np.float32).T),
        "W2": np.ascontiguousarray(np.asarray(W2, np.float32)),
        "Wc2T": np.ascontiguousarray(np.asarray(Wc2, np.float32).T),
        "b1": np.asarray(b1, np.float32).reshape(D1, 1),
        "bc1": np.asarray(bc1, np.float32).reshape(D1, 1),
        "b2": np.asarray(b2, np.float32).reshape(D2, 1),
        "bc2": np.ascontiguousarray(
            np.asarray(bc2, np.float32).reshape(DOUT // 128, 128).T
        ),
        "ident": np.eye(128, dtype=np.float32),
    }
    in_maps = []
    for c in range(NCORES):
        bs = [BPC * c + j for j in range(BPC)]
        m = dict(common)
        m["xt"] = np.ascontiguousarray(xt[bs])
        m["idx16"] = np.ascontiguousarray(wrapped[bs])
        in_maps.append(m)
    return in_maps


def kernel(_trace=False, _trace_kwargs=None, **inputs):
    nc = _get_nc()
    in_maps = _prep_inputs(**inputs)
    res = run_bass_kernel_spmd(
        nc,
        in_maps,
        list(range(NCORES)),
        trace=_trace,
        **(_trace_kwargs or {}),
    )
    out = np.empty((B, DOUT, N), np.float32)
    for c in range(NCORES):
        for j in range(BPC):
            out[BPC * c + j] = res.results[c]["out"][j]
    if _trace:
        return out, res
    return out
